# revision 1
# baseline (speedup 1.0000x reference)
"""Trainium2 Bass kernel for nn_ClassAwareLoss (class-aware frame loss).

Contract: kernel(**inputs) takes the FULL unsharded inputs (numpy arrays,
keyed as in setup_inputs()) and returns the FULL output (a float32 scalar).

Strategy (data-parallel over batch, per the sharding hint):
  - Shard `input`/`target` row-wise across 8 NeuronCores (2048 samples each).
  - Replicate the small tensors (frames^T, per-frame class ids, per-frame
    cosine weights) to every core.
  - Each core computes partial sums of
        caloss_c = sum_b sum_f [class(f)==t_b] * cosine_c[t_b] * (1 - d_bf)^2
        reg_c    = sum_b (||x_b|| - 1)^2
    and the host combines: (sum caloss + 6e-4 * sum reg) / B.

Device algorithm (per core, 2048 samples):
  dots are computed in bf16 on the PE (fp32 accumulate in PSUM); the
  normalization 1/||x|| is folded into the ScalarE pass that computes
  S = (1 - g*r)^2 via activation(Square, scale=-g, bias=1).  The
  class mask and per-frame cosine weight fuse into one DVE
  scalar_tensor_tensor op: w = (frame_class == t) * cosine_c[frame_class],
  and a tensor_tensor_reduce accumulates sum(w * S) per partition.
"""

import sys
import types
from contextlib import ExitStack

sys.path.insert(0, "/opt/trn_rl_repo")

import numpy as np
import ml_dtypes

# ---------------------------------------------------------------------------
# antenv.axon_hooks shim: lets run_bass_kernel_spmd(trace=True) capture NTFF
# profiles under axon.  Harmless when BASS_TRACE is not set.
# ---------------------------------------------------------------------------
try:
    import antenv

    if "antenv.axon_hooks" not in sys.modules:
        _mod = types.ModuleType("antenv.axon_hooks")
        _hook = [None]
        _mod.set_axon_ntff_profile_hook = lambda h: _hook.__setitem__(0, h)
        _mod.get_axon_ntff_profile_hook = lambda: _hook[0]
        sys.modules["antenv.axon_hooks"] = _mod
        antenv.axon_hooks = _mod
        try:
            from trn_agent_boot.trn_boot import _ntff_profile_via_ctypes

            _mod.set_axon_ntff_profile_hook(
                _ntff_profile_via_ctypes("/opt/axon/libaxon_pjrt.so")
            )
        except Exception:
            pass
except Exception:
    pass

import concourse.bass as bass
import concourse.tile as tile
import concourse.bass_utils as bass_utils
from concourse import bacc, mybir

# No cloud bucket in this container; keep artifacts local.
bass_utils.upload_artifacts = lambda tmpdir: "local://" + tmpdir

# ---------------------------------------------------------------------------
# Problem constants (from the reference problem definition; input-independent)
# ---------------------------------------------------------------------------
N_CORES = 8
B = 16384
D = 256
NCLS = 100
F_PARAM = 17
BS = B // N_CORES            # 2048 samples per core
NT = BS // 128               # 16 sample-tiles of 128 per core
F_TOTAL = NCLS * (F_PARAM - 1)  # 1600 frame rows

_CLS_SAMPLES = [5000 - 50 * i for i in range(100)]


def _calc_cls_idx(cls_samples, f):
    nc_ = len(cls_samples)
    n_samples = sum(cls_samples)
    ca_frame_num = [int((f - 2) * nc_ * r / n_samples) + 1 for r in cls_samples]
    over_flow = nc_ * (f - 1) - sum(ca_frame_num)
    for i in range(over_flow):
        ca_frame_num[i] += 1
    ca_frame_num.reverse()
    cls_frame_idx = [sum(ca_frame_num[0:k]) for k in range(nc_ + 1)]
    return cls_frame_idx, ca_frame_num


CLS_FRAME_IDX, CA_FRAME_NUM = _calc_cls_idx(_CLS_SAMPLES, F_PARAM)
FRAME_CLASS = np.repeat(np.arange(NCLS), CA_FRAME_NUM)  # [1600], deterministic

BF16 = mybir.dt.bfloat16
F32 = mybir.dt.float32
AF = mybir.ActivationFunctionType
ALU = mybir.AluOpType

_COMPILED = None   # (nc, meta)
LAST_RESULT = None  # BassKernelResults of the most recent run (for test.py)


def _build_program():
    """Build + compile the SPMD Bass program (one program, run on 8 cores)."""
    nc = bacc.Bacc(
        "TRN2", target_bir_lowering=False, debug=False, num_devices=N_CORES
    )

    # Per-core inputs
    x_bf = nc.dram_tensor("x_bf", [BS, D], BF16, kind="ExternalInput").ap()
    t_f32 = nc.dram_tensor("t_f32", [128, NT], F32, kind="ExternalInput").ap()
    framesT = nc.dram_tensor("framesT", [D, F_TOTAL], BF16, kind="ExternalInput").ap()
    iota_in = nc.dram_tensor("iota_mat", [128, 128], BF16, kind="ExternalInput").ap()
    cos_in = nc.dram_tensor("cosine_mat", [128, 128], BF16, kind="ExternalInput").ap()
    ct_in = nc.dram_tensor("ct_mat", [128, F_TOTAL], BF16, kind="ExternalInput").ap()
    out = nc.dram_tensor("out", [128, 2], F32, kind="ExternalOutput").ap()

    with tile.TileContext(nc) as tc:
        with ExitStack() as ctx:
            const_pool = ctx.enter_context(tc.tile_pool(name="const", bufs=1))
            work_pool = ctx.enter_context(tc.tile_pool(name="work", bufs=1))
            s_pool = ctx.enter_context(tc.tile_pool(name="s", bufs=3))
            w_pool = ctx.enter_context(tc.tile_pool(name="w", bufs=3))
            psum_pool = ctx.enter_context(
                tc.tile_pool(name="psum", bufs=2, space="PSUM")
            )
            psum_g = ctx.enter_context(
                tc.tile_pool(name="psumg", bufs=1, space="PSUM")
            )

            # ---- x transposed first: the dots matmuls gate everything ----
            xt0 = work_pool.tile([128, BS], BF16, tag="xt0")
            xt1 = work_pool.tile([128, BS], BF16, tag="xt1")
            nc.sync.dma_start_transpose(xt0[:], x_bf[:, 0:128])
            nc.scalar.dma_start_transpose(xt1[:], x_bf[:, 128:256])

            # ---- x natural layout [128, NT*D] (tile i at cols i*D..) ----
            xn = work_pool.tile([128, NT * D], BF16, tag="xn")
            nc.sync.dma_start(
                xn[:].rearrange("p (i d) -> p i d", i=NT),
                x_bf.rearrange("(i p) d -> p i d", p=128),
            )

            framesT_sb = const_pool.tile([128, 2 * F_TOTAL], BF16, tag="framesT")
            nc.sync.dma_start(framesT_sb[:, 0:F_TOTAL], framesT[0:128, :])
            nc.sync.dma_start(framesT_sb[:, F_TOTAL : 2 * F_TOTAL], framesT[128:256, :])
            iota_sb = const_pool.tile([128, 128], BF16, tag="iota")
            nc.sync.dma_start(iota_sb[:], iota_in[:])
            cos_sb = const_pool.tile([128, 128], BF16, tag="cos")
            nc.sync.dma_start(cos_sb[:], cos_in[:])
            t_sb = const_pool.tile([128, NT], F32, tag="t")
            nc.sync.dma_start(t_sb[:], t_f32[:])
            ct_sb = const_pool.tile([128, F_TOTAL], BF16, tag="ct")
            nc.sync.dma_start(ct_sb[:], ct_in[:])

            neg_one = const_pool.tile([128, 1], F32, tag="negone")
            nc.vector.memset(neg_one[:], -1.0)

            # ---- per-sample squared norms -> [128, NT] ----
            sq = work_pool.tile([128, NT], F32, tag="sq")
            sq_dump = work_pool.tile([128, D], F32, tag="sqd")
            for i in range(NT):
                nc.scalar.activation(
                    sq_dump[:],
                    xn[:, i * D : (i + 1) * D],
                    AF.Square,
                    accum_out=sq[:, i : i + 1],
                )
            # norm, 1/norm, (norm-1)^2
            norm = work_pool.tile([128, NT], F32, tag="norm")
            nc.scalar.activation(norm[:], sq[:], AF.Sqrt)
            g = work_pool.tile([128, NT], F32, tag="g")
            nc.vector.reciprocal(g[:], norm[:])
            regsq = work_pool.tile([128, NT], F32, tag="regsq")
            nc.scalar.activation(
                regsq[:], norm[:], AF.Square, bias=neg_one[:], scale=1.0
            )
            reg_col = work_pool.tile([128, 1], F32, tag="regcol")
            nc.vector.tensor_reduce(
                out=reg_col[:], in_=regsq[:], axis=mybir.AxisListType.X, op=ALU.add
            )

            # ---- main loop over sample tiles ----
            # caloss = sum_c sum_f CT[c,f] * G[c,f],
            # G[c,f] = sum_b cosine_c[t_b] * [t_b == c] * S[b,f]   (PE matmuls)
            g_ps = psum_g.tile([128, F_TOTAL], F32, tag="G")
            HALVES = [(0, 1024), (1024, F_TOTAL)]
            for i in range(NT):
                # ct_col = cosine_c[t_b]; P = ct_col * onehot(t_b)
                ct_dump = w_pool.tile([128, 128], BF16, tag="ctdump")
                ct_col = w_pool.tile([128, 1], F32, tag="ctcol")
                nc.vector.scalar_tensor_tensor(
                    out=ct_dump[:], in0=iota_sb[:], scalar=t_sb[:, i : i + 1],
                    in1=cos_sb[:], op0=ALU.is_equal, op1=ALU.mult,
                    accum_out=ct_col[:],
                )
                p_tile = w_pool.tile([128, 128], BF16, tag="p")
                nc.vector.tensor_scalar(
                    out=p_tile[:], in0=iota_sb[:],
                    scalar1=t_sb[:, i : i + 1], scalar2=ct_col[:],
                    op0=ALU.is_equal, op1=ALU.mult,
                )
                for (flo, fhi) in HALVES:
                    hw_ = fhi - flo
                    dots = psum_pool.tile([128, hw_], F32, tag="dots")
                    for c0 in range(flo, fhi, 512):
                        c1 = min(c0 + 512, fhi)
                        nc.tensor.matmul(
                            dots[:, c0 - flo : c1 - flo],
                            lhsT=xt0[:, i * 128 : (i + 1) * 128],
                            rhs=framesT_sb[:, c0:c1],
                            start=True,
                            stop=False,
                        )
                    for c0 in range(flo, fhi, 512):
                        c1 = min(c0 + 512, fhi)
                        nc.tensor.matmul(
                            dots[:, c0 - flo : c1 - flo],
                            lhsT=xt1[:, i * 128 : (i + 1) * 128],
                            rhs=framesT_sb[:, F_TOTAL + c0 : F_TOTAL + c1],
                            start=False,
                            stop=True,
                        )
                    # S = (g*r - 1)^2  (ScalarE: PSUM -> SBUF bf16)
                    s_tile = s_pool.tile([128, hw_], BF16, tag="s")
                    nc.scalar.activation(
                        s_tile[:], dots[:], AF.Square,
                        bias=neg_one[:], scale=g[:, i : i + 1],
                    )
                    # G[:, chunk] += P^T @ S
                    for c0 in range(flo, fhi, 512):
                        c1 = min(c0 + 512, fhi)
                        nc.tensor.matmul(
                            g_ps[:, c0:c1],
                            lhsT=p_tile[:],
                            rhs=s_tile[:, c0 - flo : c1 - flo],
                            start=(i == 0),
                            stop=(i == NT - 1),
                            skip_group_check=True,
                        )

            # total caloss per class-partition: sum_f CT * G
            g_dump = w_pool.tile([128, F_TOTAL], BF16, tag="gdump")
            cal_col = work_pool.tile([128, 1], F32, tag="calcol")
            nc.vector.scalar_tensor_tensor(
                out=g_dump[:], in0=ct_sb[:], scalar=1.0, in1=g_ps[:],
                op0=ALU.mult, op1=ALU.mult, accum_out=cal_col[:],
            )
            res_sb = work_pool.tile([128, 2], F32, tag="res")
            nc.vector.tensor_copy(res_sb[:, 0:1], cal_col[:])
            nc.vector.tensor_copy(res_sb[:, 1:2], reg_col[:])
            nc.sync.dma_start(out[:], res_sb[:])

    nc.compile()
    return nc


def _prepare_inputs(inputs):
    x = np.asarray(inputs["input"], dtype=np.float32)        # [B, D]
    frames = np.asarray(inputs["frames"], dtype=np.float32)  # [F, D]
    cosine_c = np.asarray(inputs["cosine_c"], dtype=np.float32)  # [NCLS]
    target = np.asarray(inputs["target"])                    # [B] int

    x_bf = x.astype(ml_dtypes.bfloat16)
    framesT = np.ascontiguousarray(frames.T).astype(ml_dtypes.bfloat16)  # [D, F]
    iota_mat = np.ascontiguousarray(
        np.broadcast_to(
            np.arange(128, dtype=np.float32).astype(ml_dtypes.bfloat16), (128, 128)
        )
    )
    cos_pad = np.zeros(128, np.float32)
    cos_pad[:NCLS] = cosine_c
    cosine_mat = np.ascontiguousarray(
        np.broadcast_to(cos_pad.astype(ml_dtypes.bfloat16), (128, 128))
    )
    ct_mat = np.zeros((128, F_TOTAL), np.float32)
    ct_mat[FRAME_CLASS, np.arange(F_TOTAL)] = 1.0
    ct_mat = ct_mat.astype(ml_dtypes.bfloat16)

    in_maps = []
    for c in range(N_CORES):
        sl = slice(c * BS, (c + 1) * BS)
        tc_ = target[sl].astype(np.float32).reshape(NT, 128).T
        # negate target? no: t values compared with fc via is_equal.
        in_maps.append(
            {
                "x_bf": np.ascontiguousarray(x_bf[sl]),
                "t_f32": np.ascontiguousarray(tc_),
                "framesT": framesT,
                "iota_mat": iota_mat,
                "cosine_mat": cosine_mat,
                "ct_mat": ct_mat,
            }
        )
    return in_maps


def kernel(**inputs):
    global _COMPILED, LAST_RESULT
    if _COMPILED is None:
        _COMPILED = _build_program()
    nc = _COMPILED

    in_maps = _prepare_inputs(inputs)
    res = bass_utils.run_bass_kernel_spmd(
        nc, in_maps, core_ids=list(range(N_CORES))
    )
    LAST_RESULT = res

    caloss = 0.0
    reg = 0.0
    for c in range(N_CORES):
        o = res.results[c]["out"].astype(np.float64)
        caloss += o[:, 0].sum()
        reg += o[:, 1].sum()
    val = (caloss + 0.0006 * reg) / B
    return np.float32(val)



# revision 10
# speedup vs baseline: 2.7276x; 2.7276x over previous
"""Trainium2 Bass kernel for nn_ClassAwareLoss (class-aware frame loss).

Contract: kernel(**inputs) takes the FULL unsharded inputs (numpy arrays,
keyed as in setup_inputs()) and returns the FULL output (a float32 scalar).

Strategy (data-parallel over batch, per the sharding hint), v2:
  The reference computes a dense [B, 1600] dots matrix, but per sample only
  the frames of its own class (<= 31 of 1600) carry nonzero weight.  We sort
  samples by class on the host (a pure permutation -- the loss is a sum over
  samples), so each 128-sample tile spans <= 2..3 classes and needs only a
  small per-tile frame block (W columns, W = max span rounded up, 64 for the
  reference distribution).  This cuts PE work ~8x and ScalarE work ~25x.

  Per core (2048 sorted samples = 16 tiles of 128):
    - One DoubleRow matmul per tile computes psum = x_tile @ [x_tile | Fb]^T:
      cols 0:128 hold the Gram matrix (diagonal = squared norms), cols
      128:128+W hold raw dots against the tile's frame block.
    - diag extract (DVE/GpSimd alternating): sq_b = Gram[b, b]
    - norm = sqrt(sq) (ScalarE), g = 1/norm (DVE), reg = sum (norm-1)^2
    - S = (g*dots - 1)^2 via one ScalarE activation (Square, scale=g, bias=-1)
    - cal_b += sum_j Wt[b, j] * S[b, j] (Wt = cosine_c[target]*classmask,
      host-built) via stt with accum, DVE/GpSimd alternating.
  Host sums the per-partition partials in float64.
"""

import sys
import types
from contextlib import ExitStack

sys.path.insert(0, "/opt/trn_rl_repo")

import numpy as np
import ml_dtypes

# ---------------------------------------------------------------------------
# antenv.axon_hooks shim: lets run_bass_kernel_spmd(trace=True) capture NTFF
# profiles under axon.  Harmless when BASS_TRACE is not set.
# ---------------------------------------------------------------------------
try:
    import antenv

    if "antenv.axon_hooks" not in sys.modules:
        _mod = types.ModuleType("antenv.axon_hooks")
        _hook = [None]
        _mod.set_axon_ntff_profile_hook = lambda h: _hook.__setitem__(0, h)
        _mod.get_axon_ntff_profile_hook = lambda: _hook[0]
        sys.modules["antenv.axon_hooks"] = _mod
        antenv.axon_hooks = _mod
        try:
            from trn_agent_boot.trn_boot import _ntff_profile_via_ctypes

            _mod.set_axon_ntff_profile_hook(
                _ntff_profile_via_ctypes("/opt/axon/libaxon_pjrt.so")
            )
        except Exception:
            pass
except Exception:
    pass

import concourse.bass as bass
import concourse.tile as tile
import concourse.bass_utils as bass_utils
from concourse import bacc, mybir

# No cloud bucket in this container; keep artifacts local.
bass_utils.upload_artifacts = lambda tmpdir: "local://" + tmpdir

# ---------------------------------------------------------------------------
# Problem constants (input-independent)
# ---------------------------------------------------------------------------
N_CORES = 8

BF16 = mybir.dt.bfloat16
FP8 = mybir.dt.float8e4
F32 = mybir.dt.float32
AF = mybir.ActivationFunctionType
ALU = mybir.AluOpType

_COMPILED = {}      # (NT, W) -> compiled Bacc
LAST_RESULT = None  # BassKernelResults of the most recent run (for test.py)


def _build_program(NT, W):
    """SPMD program: NT sample-tiles of 128 per core, W frame cols per tile."""
    CPT = 2 * (128 + W)          # block cols per tile (two D-halves)
    NG = 6                       # norm group size; must be <= psum bufs (deadlock)
    NGRP = (NT + NG - 1) // NG

    nc = bacc.Bacc(
        "TRN2", target_bir_lowering=False, debug=False, num_devices=N_CORES
    )

    blocks_in = nc.dram_tensor("blocks", [128, NT * CPT], FP8, kind="ExternalInput").ap()
    wt_in = nc.dram_tensor("wt", [128, NT * W], BF16, kind="ExternalInput").ap()
    id_in = nc.dram_tensor("ident", [128, 128], BF16, kind="ExternalInput").ap()
    out_r = nc.dram_tensor("outr", [128, 1], F32, kind="ExternalOutput").ap()
    out_c = nc.dram_tensor("outc", [1, W], F32, kind="ExternalOutput").ap()

    with tile.TileContext(nc) as tc:
        with ExitStack() as ctx:
            const_pool = ctx.enter_context(tc.tile_pool(name="const", bufs=1))
            work_pool = ctx.enter_context(tc.tile_pool(name="work", bufs=1))
            s_pool = ctx.enter_context(tc.tile_pool(name="s", bufs=4))
            p_pool = ctx.enter_context(tc.tile_pool(name="prod", bufs=4))
            d_pool = ctx.enter_context(tc.tile_pool(name="d", bufs=4))
            psum_pool = ctx.enter_context(
                tc.tile_pool(name="psum", bufs=7, space="PSUM")
            )
            psum_cal = ctx.enter_context(
                tc.tile_pool(name="psumc", bufs=1, space="PSUM")
            )

            # ---- constants / inputs to SBUF ----
            ident_sb = const_pool.tile([128, 128], BF16, tag="ident")
            nc.gpsimd.dma_start(ident_sb[:], id_in[:])
            wt_sb = const_pool.tile([128, NT * W], BF16, tag="wt")
            nc.gpsimd.dma_start(wt_sb[:], wt_in[:])
            ones_sb = const_pool.tile([128, 1], BF16, tag="ones")
            nc.vector.memset(ones_sb[:], 1.0)
            neg_one = const_pool.tile([128, 1], F32, tag="negone")
            nc.vector.memset(neg_one[:], -1.0)

            blocks_sb = const_pool.tile([128, NT * CPT], FP8, tag="blocks")
            dma_q = [nc.sync, nc.scalar]
            for j in range(NT):
                sl = slice(j * CPT, (j + 1) * CPT)
                dma_q[j % 2].dma_start(blocks_sb[:, sl], blocks_in[:, sl])

            sq_all = work_pool.tile([128, NT], F32, tag="sq")
            norm_all = work_pool.tile([128, NT], F32, tag="nm")
            g_all = work_pool.tile([128, NT], F32, tag="g")
            cal_ps = psum_cal.tile([1, W], F32, tag="calps")

            psums = {}
            for grp in range(NGRP):
                jlo = grp * NG
                jhi = min(jlo + NG, NT)
                for j in range(jlo, jhi):
                    blk = blocks_sb[:, j * CPT : (j + 1) * CPT].rearrange(
                        "p (two n) -> p two n", two=2
                    )
                    ps = psum_pool.tile([128, 128 + W], F32, tag="ps")
                    nc.tensor.matmul(
                        ps[:],
                        lhsT=blk[:, :, 0:128],
                        rhs=blk,
                        start=True,
                        stop=True,
                        perf_mode=mybir.MatmulPerfMode.DoubleRow,
                    )
                    psums[j] = ps
                    # diagonal of the Gram block -> squared norms (DVE only:
                    # GpSimd cannot read PSUM)
                    dump = d_pool.tile([128, 128], BF16, tag="dd")
                    nc.vector.scalar_tensor_tensor(
                        out=dump[:],
                        in0=ident_sb[:],
                        scalar=1.0,
                        in1=ps[:, 0:128],
                        op0=ALU.mult,
                        op1=ALU.mult,
                        accum_out=sq_all[:, j : j + 1],
                    )

                # frames were host-scaled by 16 (fp8 range); psum dots = 16*d.
                # norm16 = sqrt(256*sq) = 16*norm, g16 = 1/(16*norm) undoes it.
                gs = slice(jlo, jhi)
                nc.scalar.activation(norm_all[:, gs], sq_all[:, gs], AF.Sqrt, scale=256.0)
                nc.vector.reciprocal(g_all[:, gs], norm_all[:, gs])

                for j in range(jlo, jhi):
                    ps = psums.pop(j)
                    s_t = s_pool.tile([128, W], BF16, tag="s")
                    nc.scalar.activation(
                        s_t[:],
                        ps[:, 128 : 128 + W],
                        AF.Square,
                        bias=neg_one[:],
                        scale=g_all[:, j : j + 1],
                    )
                    prod = p_pool.tile([128, W], BF16, tag="prod")
                    nc.gpsimd.tensor_tensor(
                        out=prod[:],
                        in0=s_t[:],
                        in1=wt_sb[:, j * W : (j + 1) * W],
                        op=ALU.mult,
                    )
                    # column-sums of Wt*S accumulate over all tiles on the PE
                    nc.tensor.matmul(
                        cal_ps[:],
                        lhsT=ones_sb[:],
                        rhs=prod[:],
                        start=(j == 0),
                        stop=(j == NT - 1),
                        skip_group_check=True,
                    )

            # reg = sum over samples of (norm - 1)^2
            regdump = d_pool.tile([128, NT], F32, tag="rd")
            regcol = work_pool.tile([128, 1], F32, tag="regcol")
            nc.scalar.activation(
                regdump[:],
                norm_all[:],
                AF.Square,
                scale=0.0625,
                bias=neg_one[:],
                accum_out=regcol[:],
            )
            cal_sb = work_pool.tile([1, W], F32, tag="calsb")
            nc.vector.tensor_copy(cal_sb[:], cal_ps[:])
            nc.sync.dma_start(out_r[:], regcol[:])
            nc.sync.dma_start(out_c[:], cal_sb[:])

    nc.compile()
    return nc


def _prepare(inputs):
    """Host-side layout prep: sort by class, build per-tile blocks/weights."""
    x = np.asarray(inputs["input"], dtype=np.float32)            # [B, D]
    frames = np.asarray(inputs["frames"], dtype=np.float32)      # [F, D]
    cosine_c = np.asarray(inputs["cosine_c"], dtype=np.float32)  # [nc]
    target = np.asarray(inputs["target"]).astype(np.int64)       # [B]
    frame_class = np.asarray(inputs["frame_class"]).astype(np.int64)  # [F]

    B, D = x.shape
    assert D == 256 and B % (N_CORES * 128) == 0
    NT = B // (N_CORES * 128)

    perm = np.argsort(target, kind="stable")
    xs = x[perm].astype(ml_dtypes.float8_e4m3)
    ts = target[perm]
    fr_bf = (frames * 16.0).astype(ml_dtypes.float8_e4m3)

    # per-class frame row indices
    ncls = int(cosine_c.shape[0])
    cls_rows = [np.where(frame_class == c)[0] for c in range(ncls)]

    n_tiles = B // 128
    tile_fidx = []
    maxspan = 1
    for t in range(n_tiles):
        cls = np.unique(ts[t * 128 : (t + 1) * 128])
        fidx = np.concatenate([cls_rows[c] for c in cls])
        tile_fidx.append(fidx)
        maxspan = max(maxspan, len(fidx))
    W = max(32, -(-maxspan // 32) * 32)
    assert W <= 128, f"frame span {maxspan} too large for single-matmul layout"
    CPT = 2 * (128 + W)

    cw = cosine_c[ts]  # [B] per-sample cosine weight (sorted order)

    in_maps = []
    for c in range(N_CORES):
        blocks = np.zeros((128, NT * CPT), dtype=ml_dtypes.float8_e4m3)
        wt = np.zeros((128, NT * W), dtype=ml_dtypes.bfloat16)
        for jj in range(NT):
            t = c * NT + jj
            sl = slice(t * 128, (t + 1) * 128)
            xt = xs[sl]                      # [128, 256] bf16
            fidx = tile_fidx[t]
            base = jj * CPT
            for h in range(2):
                hb = base + h * (128 + W)
                blocks[:, hb : hb + 128] = xt[:, h * 128 : (h + 1) * 128].T
                fb = fr_bf[fidx][:, h * 128 : (h + 1) * 128]  # [nf, 128]
                blocks[:, hb + 128 : hb + 128 + len(fidx)] = fb.T
            mask = frame_class[fidx][None, :] == ts[sl][:, None]  # [128, nf]
            wt[:, jj * W : jj * W + len(fidx)] = (
                cw[sl][:, None] * mask
            ).astype(ml_dtypes.bfloat16)
        in_maps.append(
            {
                "blocks": blocks,
                "wt": wt,
                "ident": np.eye(128, dtype=ml_dtypes.bfloat16),
            }
        )
    return in_maps, NT, W


def kernel(**inputs):
    global LAST_RESULT
    in_maps, NT, W = _prepare(inputs)
    key = (NT, W)
    if key not in _COMPILED:
        _COMPILED[key] = _build_program(NT, W)
    nc = _COMPILED[key]

    res = bass_utils.run_bass_kernel_spmd(
        nc, in_maps, core_ids=list(range(N_CORES))
    )
    LAST_RESULT = res

    B = NT * 128 * N_CORES
    caloss = 0.0
    reg = 0.0
    for c in range(N_CORES):
        caloss += res.results[c]["outc"].astype(np.float64).sum()
        reg += res.results[c]["outr"].astype(np.float64).sum()
    val = (caloss + 0.0006 * reg) / B
    return np.float32(val)


# revision 11
# speedup vs baseline: 3.6972x; 1.3555x over previous
"""Trainium2 Bass kernel for nn_ClassAwareLoss (class-aware frame loss).

Contract: kernel(**inputs) takes the FULL unsharded inputs (numpy arrays,
keyed as in setup_inputs()) and returns the FULL output (a float32 scalar).

Strategy (data-parallel over batch, per the sharding hint), v2:
  The reference computes a dense [B, 1600] dots matrix, but per sample only
  the frames of its own class (<= 31 of 1600) carry nonzero weight.  We sort
  samples by class on the host (a pure permutation -- the loss is a sum over
  samples), so each 128-sample tile spans <= 2..3 classes and needs only a
  small per-tile frame block (W columns, W = max span rounded up, 64 for the
  reference distribution).  This cuts PE work ~8x and ScalarE work ~25x.

  Per core (2048 sorted samples = 16 tiles of 128):
    - One DoubleRow matmul per tile computes psum = x_tile @ [x_tile | Fb]^T:
      cols 0:128 hold the Gram matrix (diagonal = squared norms), cols
      128:128+W hold raw dots against the tile's frame block.
    - diag extract (DVE/GpSimd alternating): sq_b = Gram[b, b]
    - norm = sqrt(sq) (ScalarE), g = 1/norm (DVE), reg = sum (norm-1)^2
    - S = (g*dots - 1)^2 via one ScalarE activation (Square, scale=g, bias=-1)
    - cal_b += sum_j Wt[b, j] * S[b, j] (Wt = cosine_c[target]*classmask,
      host-built) via stt with accum, DVE/GpSimd alternating.
  Host sums the per-partition partials in float64.
"""

import sys
import types
from contextlib import ExitStack

sys.path.insert(0, "/opt/trn_rl_repo")

import numpy as np
import ml_dtypes

# ---------------------------------------------------------------------------
# antenv.axon_hooks shim: lets run_bass_kernel_spmd(trace=True) capture NTFF
# profiles under axon.  Harmless when BASS_TRACE is not set.
# ---------------------------------------------------------------------------
try:
    import antenv

    if "antenv.axon_hooks" not in sys.modules:
        _mod = types.ModuleType("antenv.axon_hooks")
        _hook = [None]
        _mod.set_axon_ntff_profile_hook = lambda h: _hook.__setitem__(0, h)
        _mod.get_axon_ntff_profile_hook = lambda: _hook[0]
        sys.modules["antenv.axon_hooks"] = _mod
        antenv.axon_hooks = _mod
        try:
            from trn_agent_boot.trn_boot import _ntff_profile_via_ctypes

            _mod.set_axon_ntff_profile_hook(
                _ntff_profile_via_ctypes("/opt/axon/libaxon_pjrt.so")
            )
        except Exception:
            pass
except Exception:
    pass

import concourse.bass as bass
import concourse.tile as tile
import concourse.bass_utils as bass_utils
from concourse import bacc, mybir

# No cloud bucket in this container; keep artifacts local.
bass_utils.upload_artifacts = lambda tmpdir: "local://" + tmpdir

# ---------------------------------------------------------------------------
# Problem constants (input-independent)
# ---------------------------------------------------------------------------
N_CORES = 8

BF16 = mybir.dt.bfloat16
FP8 = mybir.dt.float8e4
F32 = mybir.dt.float32
AF = mybir.ActivationFunctionType
ALU = mybir.AluOpType

_COMPILED = {}      # (NT, W) -> compiled Bacc
LAST_RESULT = None  # BassKernelResults of the most recent run (for test.py)


def _build_program(NT, W):
    """SPMD program: NT sample-tiles of 128 per core, W frame cols per tile."""
    CPT = 2 * (128 + W)          # block cols per tile (two D-halves)
    NG = 4                       # norm/quad group size; must be <= psum bufs
    NGRP = (NT + NG - 1) // NG
    QW = NG * W                  # quad product width

    nc = bacc.Bacc(
        "TRN2", target_bir_lowering=False, debug=False, num_devices=N_CORES
    )

    blocks_in = nc.dram_tensor("blocks", [128, NT * CPT], FP8, kind="ExternalInput").ap()
    wt_in = nc.dram_tensor("wt", [128, NT * W], BF16, kind="ExternalInput").ap()
    id_in = nc.dram_tensor("ident", [128, 128], BF16, kind="ExternalInput").ap()
    out_c = nc.dram_tensor("outc", [1, QW + NT], F32, kind="ExternalOutput").ap()

    with tile.TileContext(nc) as tc:
        with ExitStack() as ctx:
            const_pool = ctx.enter_context(tc.tile_pool(name="const", bufs=1))
            rot_pool = ctx.enter_context(tc.tile_pool(name="rot", bufs=3))
            psum_pool = ctx.enter_context(
                tc.tile_pool(name="psum", bufs=7, space="PSUM")
            )
            psum_cal = ctx.enter_context(
                tc.tile_pool(name="psumc", bufs=1, space="PSUM")
            )

            ones_sb = const_pool.tile([128, 1], BF16, tag="ones")
            nc.vector.memset(ones_sb[:], 1.0)
            neg_one = const_pool.tile([128, 1], F32, tag="negone")
            nc.vector.memset(neg_one[:], -1.0)
            # prefetch both ScalarE activation tables during the DMA fill
            dummy = const_pool.tile([128, 1], F32, tag="dumy")
            nc.scalar.activation(dummy[:], neg_one[:], AF.Sqrt, scale=-1.0)
            nc.scalar.activation(dummy[:], neg_one[:], AF.Square)

            # wt/ident via the gpsimd software DGE; blocks chunked on the SP
            # hardware queue (keeps the ScalarE queue free for activations)
            ident_sb = const_pool.tile([128, 128], BF16, tag="ident")
            nc.gpsimd.dma_start(ident_sb[:], id_in[:])
            wt_sb = const_pool.tile([128, NT * W], BF16, tag="wt")
            nc.gpsimd.dma_start(wt_sb[:], wt_in[:])

            blocks_sb = const_pool.tile([128, NT * CPT], FP8, tag="blocks")
            for g in range(NGRP):
                sl = slice(g * NG * CPT, min((g + 1) * NG, NT) * CPT)
                nc.sync.dma_start(blocks_sb[:, sl], blocks_in[:, sl])

            sq_all = const_pool.tile([128, NT], F32, tag="sq")
            norm_all = const_pool.tile([128, NT], F32, tag="nm")
            g_all = const_pool.tile([128, NT], F32, tag="g")
            cal_ps = psum_cal.tile([1, QW + NT], F32, tag="calps")

            for grp in range(NGRP):
                jlo = grp * NG
                jhi = min(jlo + NG, NT)
                psums = []
                for j in range(jlo, jhi):
                    blk = blocks_sb[:, j * CPT : (j + 1) * CPT].rearrange(
                        "p (two n) -> p two n", two=2
                    )
                    ps = psum_pool.tile([128, 128 + W], F32, tag="ps")
                    nc.tensor.matmul(
                        ps[:],
                        lhsT=blk[:, :, 0:128],
                        rhs=blk,
                        start=True,
                        stop=True,
                        perf_mode=mybir.MatmulPerfMode.DoubleRow,
                    )
                    psums.append(ps)
                    # diagonal of the Gram block -> squared norms (DVE only:
                    # GpSimd cannot read PSUM)
                    dump = rot_pool.tile([128, 128], BF16, tag="dd")
                    nc.vector.scalar_tensor_tensor(
                        out=dump[:],
                        in0=ident_sb[:],
                        scalar=1.0,
                        in1=ps[:, 0:128],
                        op0=ALU.mult,
                        op1=ALU.mult,
                        accum_out=sq_all[:, j : j + 1],
                    )

                # frames were host-scaled by 16 (fp8 range); psum dots = 16*d.
                # norm16 = sqrt(256*sq) = 16*norm, g16 = 1/(16*norm) undoes it.
                gs = slice(jlo, jhi)
                nc.scalar.activation(norm_all[:, gs], sq_all[:, gs], AF.Sqrt, scale=256.0)
                nc.vector.reciprocal(g_all[:, gs], norm_all[:, gs])

                s_q = rot_pool.tile([128, QW], BF16, tag="s")
                for j in range(jlo, jhi):
                    ps = psums[j - jlo]
                    nc.scalar.activation(
                        s_q[:, (j - jlo) * W : (j - jlo + 1) * W],
                        ps[:, 128 : 128 + W],
                        AF.Square,
                        bias=neg_one[:],
                        scale=g_all[:, j : j + 1],
                    )
                prod = rot_pool.tile([128, QW], BF16, tag="prod")
                nc.gpsimd.tensor_tensor(
                    out=prod[:],
                    in0=s_q[:],
                    in1=wt_sb[:, jlo * W : jhi * W],
                    op=ALU.mult,
                )
                # column-sums of Wt*S accumulate over all quads on the PE
                nc.tensor.matmul(
                    cal_ps[:, 0:QW],
                    lhsT=ones_sb[:],
                    rhs=prod[:],
                    start=(grp == 0),
                    stop=(grp == NGRP - 1),
                    skip_group_check=True,
                )

            # reg = sum_b (norm - 1)^2, column-reduced on the PE as well
            regdump = rot_pool.tile([128, NT], BF16, tag="rd")
            nc.scalar.activation(
                regdump[:],
                norm_all[:],
                AF.Square,
                scale=0.0625,
                bias=neg_one[:],
            )
            nc.tensor.matmul(
                cal_ps[:, QW : QW + NT],
                lhsT=ones_sb[:],
                rhs=regdump[:],
                start=True,
                stop=True,
                skip_group_check=True,
            )
            cal_sb = const_pool.tile([1, QW + NT], F32, tag="calsb")
            nc.vector.tensor_copy(cal_sb[:], cal_ps[:])
            nc.sync.dma_start(out_c[:], cal_sb[:])

    nc.compile()
    return nc


def _prepare(inputs):
    """Host-side layout prep: sort by class, build per-tile blocks/weights."""
    x = np.asarray(inputs["input"], dtype=np.float32)            # [B, D]
    frames = np.asarray(inputs["frames"], dtype=np.float32)      # [F, D]
    cosine_c = np.asarray(inputs["cosine_c"], dtype=np.float32)  # [nc]
    target = np.asarray(inputs["target"]).astype(np.int64)       # [B]
    frame_class = np.asarray(inputs["frame_class"]).astype(np.int64)  # [F]

    B, D = x.shape
    assert D == 256 and B % (N_CORES * 128) == 0
    NT = B // (N_CORES * 128)

    perm = np.argsort(target, kind="stable")
    xs = x[perm].astype(ml_dtypes.float8_e4m3)
    ts = target[perm]
    fr_bf = (frames * 16.0).astype(ml_dtypes.float8_e4m3)

    # per-class frame row indices
    ncls = int(cosine_c.shape[0])
    cls_rows = [np.where(frame_class == c)[0] for c in range(ncls)]

    n_tiles = B // 128
    tile_fidx = []
    maxspan = 1
    for t in range(n_tiles):
        cls = np.unique(ts[t * 128 : (t + 1) * 128])
        fidx = np.concatenate([cls_rows[c] for c in cls])
        tile_fidx.append(fidx)
        maxspan = max(maxspan, len(fidx))
    W = max(32, -(-maxspan // 32) * 32)
    assert W <= 128, f"frame span {maxspan} too large for single-matmul layout"
    CPT = 2 * (128 + W)

    cw = cosine_c[ts]  # [B] per-sample cosine weight (sorted order)

    in_maps = []
    for c in range(N_CORES):
        blocks = np.zeros((128, NT * CPT), dtype=ml_dtypes.float8_e4m3)
        wt = np.zeros((128, NT * W), dtype=ml_dtypes.bfloat16)
        for jj in range(NT):
            t = c * NT + jj
            sl = slice(t * 128, (t + 1) * 128)
            xt = xs[sl]                      # [128, 256] bf16
            fidx = tile_fidx[t]
            base = jj * CPT
            for h in range(2):
                hb = base + h * (128 + W)
                blocks[:, hb : hb + 128] = xt[:, h * 128 : (h + 1) * 128].T
                fb = fr_bf[fidx][:, h * 128 : (h + 1) * 128]  # [nf, 128]
                blocks[:, hb + 128 : hb + 128 + len(fidx)] = fb.T
            mask = frame_class[fidx][None, :] == ts[sl][:, None]  # [128, nf]
            wt[:, jj * W : jj * W + len(fidx)] = (
                cw[sl][:, None] * mask
            ).astype(ml_dtypes.bfloat16)
        in_maps.append(
            {
                "blocks": blocks,
                "wt": wt,
                "ident": np.eye(128, dtype=ml_dtypes.bfloat16),
            }
        )
    return in_maps, NT, W


def kernel(**inputs):
    global LAST_RESULT
    in_maps, NT, W = _prepare(inputs)
    key = (NT, W)
    if key not in _COMPILED:
        _COMPILED[key] = _build_program(NT, W)
    nc = _COMPILED[key]

    res = bass_utils.run_bass_kernel_spmd(
        nc, in_maps, core_ids=list(range(N_CORES))
    )
    LAST_RESULT = res

    B = NT * 128 * N_CORES
    QW = 4 * W
    caloss = 0.0
    reg = 0.0
    for c in range(N_CORES):
        o = res.results[c]["outc"].astype(np.float64)
        caloss += o[0, 0:QW].sum()
        reg += o[0, QW : QW + NT].sum()
    val = (caloss + 0.0006 * reg) / B
    return np.float32(val)


# revision 13
# speedup vs baseline: 3.7113x; 1.0038x over previous
"""Trainium2 Bass kernel for nn_ClassAwareLoss (class-aware frame loss).

Contract: kernel(**inputs) takes the FULL unsharded inputs (numpy arrays,
keyed as in setup_inputs()) and returns the FULL output (a float32 scalar).

Strategy (data-parallel over batch, per the sharding hint), v2:
  The reference computes a dense [B, 1600] dots matrix, but per sample only
  the frames of its own class (<= 31 of 1600) carry nonzero weight.  We sort
  samples by class on the host (a pure permutation -- the loss is a sum over
  samples), so each 128-sample tile spans <= 2..3 classes and needs only a
  small per-tile frame block (W columns, W = max span rounded up, 64 for the
  reference distribution).  This cuts PE work ~8x and ScalarE work ~25x.

  Per core (2048 sorted samples = 16 tiles of 128):
    - One DoubleRow matmul per tile computes psum = x_tile @ [x_tile | Fb]^T:
      cols 0:128 hold the Gram matrix (diagonal = squared norms), cols
      128:128+W hold raw dots against the tile's frame block.
    - diag extract (DVE/GpSimd alternating): sq_b = Gram[b, b]
    - norm = sqrt(sq) (ScalarE), g = 1/norm (DVE), reg = sum (norm-1)^2
    - S = (g*dots - 1)^2 via one ScalarE activation (Square, scale=g, bias=-1)
    - cal_b += sum_j Wt[b, j] * S[b, j] (Wt = cosine_c[target]*classmask,
      host-built) via stt with accum, DVE/GpSimd alternating.
  Host sums the per-partition partials in float64.
"""

import sys
import types
from contextlib import ExitStack

sys.path.insert(0, "/opt/trn_rl_repo")

import numpy as np
import ml_dtypes

# ---------------------------------------------------------------------------
# antenv.axon_hooks shim: lets run_bass_kernel_spmd(trace=True) capture NTFF
# profiles under axon.  Harmless when BASS_TRACE is not set.
# ---------------------------------------------------------------------------
try:
    import antenv

    if "antenv.axon_hooks" not in sys.modules:
        _mod = types.ModuleType("antenv.axon_hooks")
        _hook = [None]
        _mod.set_axon_ntff_profile_hook = lambda h: _hook.__setitem__(0, h)
        _mod.get_axon_ntff_profile_hook = lambda: _hook[0]
        sys.modules["antenv.axon_hooks"] = _mod
        antenv.axon_hooks = _mod
        try:
            from trn_agent_boot.trn_boot import _ntff_profile_via_ctypes

            _mod.set_axon_ntff_profile_hook(
                _ntff_profile_via_ctypes("/opt/axon/libaxon_pjrt.so")
            )
        except Exception:
            pass
except Exception:
    pass

import concourse.bass as bass
import concourse.tile as tile
import concourse.bass_utils as bass_utils
from concourse import bacc, mybir

# No cloud bucket in this container; keep artifacts local.
bass_utils.upload_artifacts = lambda tmpdir: "local://" + tmpdir

# ---------------------------------------------------------------------------
# Problem constants (input-independent)
# ---------------------------------------------------------------------------
N_CORES = 8

BF16 = mybir.dt.bfloat16
FP8 = mybir.dt.float8e4
F32 = mybir.dt.float32
AF = mybir.ActivationFunctionType
ALU = mybir.AluOpType

_COMPILED = {}      # (NT, W) -> compiled Bacc
LAST_RESULT = None  # BassKernelResults of the most recent run (for test.py)


def _build_program(NT, W):
    """SPMD program: NT sample-tiles of 128 per core, W frame cols per tile."""
    CPT = 2 * (128 + W)          # block cols per tile (two D-halves)
    NG = 4                       # norm/quad group size; must be <= psum bufs
    NGRP = (NT + NG - 1) // NG
    QW = NG * W                  # quad product width

    nc = bacc.Bacc(
        "TRN2", target_bir_lowering=False, debug=False, num_devices=N_CORES
    )

    blocks_in = nc.dram_tensor("blocks", [128, NT * CPT], FP8, kind="ExternalInput").ap()
    wt_in = nc.dram_tensor("wt", [128, NT * W], BF16, kind="ExternalInput").ap()
    id_in = nc.dram_tensor("ident", [128, 128], BF16, kind="ExternalInput").ap()
    out_c = nc.dram_tensor("outc", [1, QW + NT], F32, kind="ExternalOutput").ap()

    with tile.TileContext(nc) as tc:
        with ExitStack() as ctx:
            const_pool = ctx.enter_context(tc.tile_pool(name="const", bufs=1))
            rot_pool = ctx.enter_context(tc.tile_pool(name="rot", bufs=3))
            psum_pool = ctx.enter_context(
                tc.tile_pool(name="psum", bufs=7, space="PSUM")
            )
            psum_cal = ctx.enter_context(
                tc.tile_pool(name="psumc", bufs=1, space="PSUM")
            )
            prod_pool = ctx.enter_context(tc.tile_pool(name="prodp", bufs=4))

            ones_sb = const_pool.tile([128, 1], BF16, tag="ones")
            nc.vector.memset(ones_sb[:], 1.0)
            neg_one = const_pool.tile([128, 1], F32, tag="negone")
            nc.vector.memset(neg_one[:], -1.0)
            # prefetch both ScalarE activation tables during the DMA fill
            dummy = const_pool.tile([128, 1], F32, tag="dumy")
            nc.scalar.activation(dummy[:], neg_one[:], AF.Sqrt, scale=-1.0)
            nc.scalar.activation(dummy[:], neg_one[:], AF.Square)

            # wt/ident via the gpsimd software DGE; blocks chunked on the SP
            # hardware queue (keeps the ScalarE queue free for activations)
            ident_sb = const_pool.tile([128, 128], BF16, tag="ident")
            nc.gpsimd.dma_start(ident_sb[:], id_in[:])
            wt_sb = const_pool.tile([128, NT * W], BF16, tag="wt")
            nc.gpsimd.dma_start(wt_sb[:], wt_in[:])

            blocks_sb = const_pool.tile([128, NT * CPT], FP8, tag="blocks")
            for g in range(NGRP):
                sl = slice(g * NG * CPT, min((g + 1) * NG, NT) * CPT)
                nc.sync.dma_start(blocks_sb[:, sl], blocks_in[:, sl])

            sq_all = const_pool.tile([128, NT], F32, tag="sq")
            norm_all = const_pool.tile([128, NT], F32, tag="nm")
            g_all = const_pool.tile([128, NT], F32, tag="g")
            cal_ps = psum_cal.tile([1, QW + NT], F32, tag="calps")

            prods = []
            for grp in range(NGRP):
                jlo = grp * NG
                jhi = min(jlo + NG, NT)
                psums = []
                for j in range(jlo, jhi):
                    blk = blocks_sb[:, j * CPT : (j + 1) * CPT].rearrange(
                        "p (two n) -> p two n", two=2
                    )
                    ps = psum_pool.tile([128, 128 + W], F32, tag="ps")
                    nc.tensor.matmul(
                        ps[:],
                        lhsT=blk[:, :, 0:128],
                        rhs=blk,
                        start=True,
                        stop=True,
                        perf_mode=mybir.MatmulPerfMode.DoubleRow,
                    )
                    psums.append(ps)
                    # diagonal of the Gram block -> squared norms (DVE only:
                    # GpSimd cannot read PSUM)
                    dump = rot_pool.tile([128, 128], BF16, tag="dd")
                    nc.vector.scalar_tensor_tensor(
                        out=dump[:],
                        in0=ident_sb[:],
                        scalar=1.0,
                        in1=ps[:, 0:128],
                        op0=ALU.mult,
                        op1=ALU.mult,
                        accum_out=sq_all[:, j : j + 1],
                    )

                # frames were host-scaled by 16 (fp8 range); psum dots = 16*d.
                # norm16 = sqrt(256*sq) = 16*norm, g16 = 1/(16*norm) undoes it.
                gs = slice(jlo, jhi)
                nc.scalar.activation(norm_all[:, gs], sq_all[:, gs], AF.Sqrt, scale=256.0)
                nc.vector.reciprocal(g_all[:, gs], norm_all[:, gs])

                s_q = rot_pool.tile([128, QW], BF16, tag="s")
                for j in range(jlo, jhi):
                    ps = psums[j - jlo]
                    nc.scalar.activation(
                        s_q[:, (j - jlo) * W : (j - jlo + 1) * W],
                        ps[:, 128 : 128 + W],
                        AF.Square,
                        bias=neg_one[:],
                        scale=g_all[:, j : j + 1],
                    )
                prod = prod_pool.tile([128, QW], BF16, tag=f"prod{grp}")
                nc.gpsimd.tensor_tensor(
                    out=prod[:],
                    in0=s_q[:],
                    in1=wt_sb[:, jlo * W : jhi * W],
                    op=ALU.mult,
                )
                prods.append(prod)

            # column-sums of Wt*S accumulate over all quads on the PE; these
            # run after every big matmul so the PE stream is stall-free
            for grp, prod in enumerate(prods):
                nc.tensor.matmul(
                    cal_ps[:, 0:QW],
                    lhsT=ones_sb[:],
                    rhs=prod[:],
                    start=(grp == 0),
                    stop=(grp == len(prods) - 1),
                    skip_group_check=True,
                )

            # reg = sum_b (norm - 1)^2, column-reduced on the PE as well
            regdump = rot_pool.tile([128, NT], BF16, tag="rd")
            nc.scalar.activation(
                regdump[:],
                norm_all[:],
                AF.Square,
                scale=0.0625,
                bias=neg_one[:],
            )
            nc.tensor.matmul(
                cal_ps[:, QW : QW + NT],
                lhsT=ones_sb[:],
                rhs=regdump[:],
                start=True,
                stop=True,
                skip_group_check=True,
            )
            cal_sb = const_pool.tile([1, QW + NT], F32, tag="calsb")
            nc.vector.tensor_copy(cal_sb[:], cal_ps[:])
            nc.sync.dma_start(out_c[:], cal_sb[:])

    nc.compile()
    return nc


def _prepare(inputs):
    """Host-side layout prep: sort by class, build per-tile blocks/weights."""
    x = np.asarray(inputs["input"], dtype=np.float32)            # [B, D]
    frames = np.asarray(inputs["frames"], dtype=np.float32)      # [F, D]
    cosine_c = np.asarray(inputs["cosine_c"], dtype=np.float32)  # [nc]
    target = np.asarray(inputs["target"]).astype(np.int64)       # [B]
    frame_class = np.asarray(inputs["frame_class"]).astype(np.int64)  # [F]

    B, D = x.shape
    assert D == 256 and B % (N_CORES * 128) == 0
    NT = B // (N_CORES * 128)

    perm = np.argsort(target, kind="stable")
    xs = x[perm].astype(ml_dtypes.float8_e4m3)
    ts = target[perm]
    fr_bf = (frames * 16.0).astype(ml_dtypes.float8_e4m3)

    # per-class frame row indices
    ncls = int(cosine_c.shape[0])
    cls_rows = [np.where(frame_class == c)[0] for c in range(ncls)]

    n_tiles = B // 128
    tile_fidx = []
    maxspan = 1
    for t in range(n_tiles):
        cls = np.unique(ts[t * 128 : (t + 1) * 128])
        fidx = np.concatenate([cls_rows[c] for c in cls])
        tile_fidx.append(fidx)
        maxspan = max(maxspan, len(fidx))
    W = max(32, -(-maxspan // 32) * 32)
    assert W <= 128, f"frame span {maxspan} too large for single-matmul layout"
    CPT = 2 * (128 + W)

    cw = cosine_c[ts]  # [B] per-sample cosine weight (sorted order)

    in_maps = []
    for c in range(N_CORES):
        blocks = np.zeros((128, NT * CPT), dtype=ml_dtypes.float8_e4m3)
        wt = np.zeros((128, NT * W), dtype=ml_dtypes.bfloat16)
        for jj in range(NT):
            t = c * NT + jj
            sl = slice(t * 128, (t + 1) * 128)
            xt = xs[sl]                      # [128, 256] bf16
            fidx = tile_fidx[t]
            base = jj * CPT
            for h in range(2):
                hb = base + h * (128 + W)
                blocks[:, hb : hb + 128] = xt[:, h * 128 : (h + 1) * 128].T
                fb = fr_bf[fidx][:, h * 128 : (h + 1) * 128]  # [nf, 128]
                blocks[:, hb + 128 : hb + 128 + len(fidx)] = fb.T
            mask = frame_class[fidx][None, :] == ts[sl][:, None]  # [128, nf]
            wt[:, jj * W : jj * W + len(fidx)] = (
                cw[sl][:, None] * mask
            ).astype(ml_dtypes.bfloat16)
        in_maps.append(
            {
                "blocks": blocks,
                "wt": wt,
                "ident": np.eye(128, dtype=ml_dtypes.bfloat16),
            }
        )
    return in_maps, NT, W


def kernel(**inputs):
    global LAST_RESULT
    in_maps, NT, W = _prepare(inputs)
    key = (NT, W)
    if key not in _COMPILED:
        _COMPILED[key] = _build_program(NT, W)
    nc = _COMPILED[key]

    res = bass_utils.run_bass_kernel_spmd(
        nc, in_maps, core_ids=list(range(N_CORES))
    )
    LAST_RESULT = res

    B = NT * 128 * N_CORES
    QW = 4 * W
    caloss = 0.0
    reg = 0.0
    for c in range(N_CORES):
        o = res.results[c]["outc"].astype(np.float64)
        caloss += o[0, 0:QW].sum()
        reg += o[0, QW : QW + NT].sum()
    val = (caloss + 0.0006 * reg) / B
    return np.float32(val)
